# revision 25
# baseline (speedup 1.0000x reference)
"""Trainium2 Bass kernel for nn_Decoder_TRANSFORMER_14791867367496.

The reference decoder is affine in the positions: each frame step is
    pos_{t+1} = pos_t @ M + (d_t[b] + g[b,j]),   M = I + W_pe @ W3  (3x3)
(with W_final = [W1; W2; W3] split along its 768 input rows), so the whole
60-step scan has a closed form

    out[b, j, :, t] = X[b, j, :] @ Q_t + r_t[b, :]

where X = initial_grid,
    Q_t = M^t + (W_pe @ W2) @ S_t,          S_t = sum_{k<t} M^k
    r_t[b] = h @ S_t + D_t[b],              D_t = sum_{s=1..t} d_s M^{t-s}
    d_t[b] = (emb_table[t] + z @ W_clip + b_clip) @ W1
    h      = b_pe @ (W2 + W3) + b_final

All of Q/r are tiny (3x3 / per-batch 3-vectors) and are computed on the host
in float64.  The device kernel is then a single affine map per point
([3 feats -> 180 outputs] + per-batch bias) and is bandwidth/stream bound.

Device arithmetic: fp8(e4m3) DoubleRow matmuls.  Each operand is split into
three e4m3 chunks (x = x0 + x1/16 + x2/256, ~4 mantissa bits each); all
cross terms with a+b <= 2 are kept (error ~2^-12) plus 3 e4m3 bias-chunk
rows, giving a 21-row K-stack per point-tile.  Per-row power-of-two scales
keep both factors inside e4m3 range; fp8 products are exact in the f32 PSUM
accumulation (PE upcasts to e6m3/e10m10).  DoubleRow packs the two tiles of
a pair as the two K-planes, so the PE streams 2 psum columns/cycle — half
the streaming time of the bf16 formulation.  Measured end-to-end error vs
the f32 reference is ~2.1e-4 (dominated by the fp16 output rounding).

Output is written as fp16 (norm rel err 2.1e-4 vs 2e-2 budget), halving the
HBM write traffic; the host widens back to f32 while unsharding.

Schedule (per core, 4 batches x 4096 points = 128 point-tiles = 64 pair
matmuls into the 8 psum banks): PE streams matmuls; DVE+ACT drain psum in
4-bank copies (f32 -> fp16, 7/9 split by engine speed) into a flat 46KB
stage; SP issues 16 half-group output DMAs as the copies land.  Inputs are
chunked so the first matmul's operands arrive with minimum latency (2-mm
first chunk on SP ring, rhs on ACT ring).  The fixed ~6.5us NEFF epilogue
(per-engine semaphore-range walk) overlaps the output-DMA drain: the end of
the kernel is max(last DMA + epsilon, last copy + epilogue).

Sharding: data-parallel over batch — each of the 8 cores handles 4 batches.
"""

import numpy as np

BS, NFRAMES, NJOINTS, NFEATS, LATENT, CLIP = 32, 60, 4096, 3, 256, 512
NCORES = 8
B_PER_CORE = BS // NCORES                  # 4
PTS = B_PER_CORE * NJOINTS                 # 16384 points per core
NTILES = PTS // 128                        # 128 point-tiles per core
GROUPS = 8                                 # output groups (1 psum cycle each)
TPG = NTILES // GROUPS                     # 16 tiles per group
FC = NFEATS * NFRAMES                      # 180 output columns per point
KR = 21                                    # K rows: 18 products + 3 bias
PAIR = 2                                   # tiles fused per matmul (planes)
MM = NTILES // PAIR                        # 64 matmuls
MM_PER_G = TPG // PAIR                     # 8 matmuls per group
XCH = [0, 0, 1, 0, 1, 2]                   # x-chunk index per product row
QCH = [0, 1, 0, 2, 1, 0]                   # q-chunk index per product row
# psum->stage copies as (first matmul covered, n matmuls): two-bank copies
# except the last four, which are single-bank so the final copy lands right
# after the final matmul (shortens the pre-epilogue tail).  Copies strictly
# alternate DVE/ACT so the engines always overlap and the 8-bank psum gives
# 4 copies of lookahead — PE never stalls on drains.
COPIES = [(2 * _c, 2) for _c in range(30)] + [(60, 1), (61, 1), (62, 1), (63, 1)]
NCOPY = len(COPIES)
ENG = ['v' if _c % 2 == 0 else 'a' for _c in range(NCOPY)]
POS = [_c // 2 + 1 for _c in range(NCOPY)]  # 1-based position within engine


def _precompute(z, W_pe, b_pe, W_clip, b_clip, emb_table, W_final, b_final):
    """Host-side f64 computation of the closed-form coefficients.

    Returns Q_all [3, 180] and r_all [32, 180], column layout c = f*60 + t
    (matching the [.., 3, 60] innermost layout of the output)."""
    f64 = np.float64
    W_pe64 = np.asarray(W_pe, f64)
    W_fin = np.asarray(W_final, f64)
    W1, W2, W3 = W_fin[:LATENT], W_fin[LATENT:2 * LATENT], W_fin[2 * LATENT:]
    M = np.eye(3) + W_pe64 @ W3
    Gm = W_pe64 @ W2
    b_pe64 = np.asarray(b_pe, f64)
    h = b_pe64 @ W2 + b_pe64 @ W3 + np.asarray(b_final, f64)
    z_proj = np.asarray(z, f64) @ np.asarray(W_clip, f64) + np.asarray(b_clip, f64)
    d = (np.asarray(emb_table, f64)[None, :, :] + z_proj[:, None, :]) @ W1  # [32,60,3]

    Q = np.zeros((NFRAMES, 3, 3))
    R = np.zeros((NFRAMES, BS, 3))
    Q[0] = np.eye(3)
    Mt = np.eye(3)
    S = np.zeros((3, 3))
    D = np.zeros((BS, 3))
    for t in range(1, NFRAMES):
        S = S + Mt
        Mt = Mt @ M
        D = D @ M + d[:, t, :]
        Q[t] = Mt + Gm @ S
        R[t] = h @ S + D
    Q_all = Q.transpose(1, 2, 0).reshape(3, FC)     # [k, f*60+t]
    r_all = R.transpose(1, 2, 0).reshape(BS, FC)    # [b, f*60+t]
    return Q_all, r_all


def _rnd8(a):
    """Round f64 to the TRN e4m3 grid (max normal 240), back as f64."""
    import ml_dtypes
    return np.clip(a, -240.0, 240.0).astype(ml_dtypes.float8_e4m3).astype(np.float64)


def _chunk3(a, scales=(1.0, 16.0, 256.0)):
    """Three e4m3 chunks with power-of-two residual scales: a ~ sum cj/sj."""
    c0 = _rnd8(a * scales[0]) / scales[0]
    c1 = _rnd8((a - c0) * scales[1]) / scales[1]
    c2 = _rnd8((a - c0 - c1) * scales[2]) / scales[2]
    return [c0 * scales[0], c1 * scales[1], c2 * scales[2]]  # stored values


def _build_bass():
    import concourse.mybir as mybir
    from concourse import bacc
    from concourse.bass import ts

    f32 = mybir.dt.float32
    f16 = mybir.dt.float16
    f8 = mybir.dt.float8e4
    DR = mybir.MatmulPerfMode.DoubleRow
    nc = bacc.Bacc(None, target_bir_lowering=False)
    xt = nc.dram_tensor("xt", [KR, MM * PAIR * 128], f8, kind="ExternalInput")
    rhs = nc.dram_tensor("rhs", [KR, B_PER_CORE * PAIR * PAIR * FC], f8,
                         kind="ExternalInput")
    out = nc.dram_tensor("out", [PTS, FC], f16, kind="ExternalOutput")
    # point p = g*2048 + j*16 + (h*8 + w): DMA h of group g writes 8 tiles
    out4 = out[:].rearrange("(g j h w) c -> g h j (w c)", g=GROUPS, j=128,
                            h=2, w=TPG // 2)

    from contextlib import ExitStack
    ctx = ExitStack()
    xt_sb = ctx.enter_context(
        nc.sbuf_tensor("xt_sb", [KR, MM * PAIR * 128], f8))
    rhs_sb = ctx.enter_context(
        nc.sbuf_tensor("rhs_sb", [KR, B_PER_CORE * PAIR * PAIR * FC], f8))
    stage = ctx.enter_context(
        nc.sbuf_tensor("stage", [128, NTILES * FC], f16))
    psum = ctx.enter_context(nc.psum_tensor("ps", [128, 8 * 512], f32))
    psum_v = psum[:].rearrange("p (bk w) -> p bk w", bk=8)

    s_c0a = ctx.enter_context(nc.semaphore("s_c0a"))
    s_c0b = ctx.enter_context(nc.semaphore("s_c0b"))
    s_rhs0 = ctx.enter_context(nc.semaphore("s_rhs0"))
    s_rhs123 = ctx.enter_context(nc.semaphore("s_rhs123"))
    s_ch1 = ctx.enter_context(nc.semaphore("s_ch1"))
    s_ch23 = ctx.enter_context(nc.semaphore("s_ch23"))
    s_ch47 = ctx.enter_context(nc.semaphore("s_ch47"))
    s_pe = ctx.enter_context(nc.semaphore("s_pe"))
    s_cpv = ctx.enter_context(nc.semaphore("s_cpv"))
    s_cpa = ctx.enter_context(nc.semaphore("s_cpa"))
    s_out = ctx.enter_context(nc.semaphore("s_out"))

    # ---- input DMAs ----
    # Matmul 0's operands are issued concurrently and first on the SP ring
    # (first-batch rhs — SP's sole early job) and the ACT ring (first 4
    # matmuls' xt).  Every bulk chunk is ordered or gated behind those so
    # its descriptors queue AFTER the critical ones in the shared 16
    # hardware DMA queues (per-queue FIFO, no priority) — ungated, bulk
    # descriptors delay matmul 0 by ~2us.
    RW = PAIR * PAIR * FC                  # 720 rhs columns per local batch
    nc.sync.dma_start(out=rhs_sb[:, :RW], in_=rhs[:, :RW]).then_inc(s_rhs0, 16)
    nc.scalar.dma_start(out=xt_sb[:, :1024], in_=xt[:, :1024]).then_inc(s_c0a, 16)
    nc.gpsimd.dma_start(out=xt_sb[:, 1024:2048],
                        in_=xt[:, 1024:2048]).then_inc(s_c0b, 16)
    # cross-core dampening: hold this core's bulk descriptors until its
    # critical rhs0 completed, so fast cores' bulk traffic doesn't flood the
    # shared queues while slow cores' first chunks are still draining
    nc.scalar.wait_ge(s_rhs0, 16)
    nc.scalar.dma_start(out=rhs_sb[:, RW:], in_=rhs[:, RW:]).then_inc(s_rhs123, 16)
    nc.scalar.dma_start(out=xt_sb[:, 2048:4096],
                        in_=xt[:, 2048:4096]).then_inc(s_ch1, 16)
    nc.sync.wait_ge(s_c0a, 16)
    nc.sync.dma_start(out=xt_sb[:, 4096:8192],
                      in_=xt[:, 4096:8192]).then_inc(s_ch23, 16)
    nc.sync.dma_start(out=xt_sb[:, 8192:],
                      in_=xt[:, 8192:]).then_inc(s_ch47, 16)

    # ---- PE: 64 DoubleRow pair-matmuls ----
    for m in range(MM):
        if m == 0:
            nc.tensor.wait_ge(s_c0a, 16)
            nc.tensor.wait_ge(s_rhs0, 16)
        elif m == 4:
            nc.tensor.wait_ge(s_c0b, 16)
        elif m == 8:
            nc.tensor.wait_ge(s_ch1, 16)
        elif m == 16:
            nc.tensor.wait_ge(s_ch23, 16)
            nc.tensor.wait_ge(s_rhs123, 16)
        elif m == 32:
            nc.tensor.wait_ge(s_ch47, 16)
        if m >= 8 and m % 2 == 0:
            cprev = (m - 8) // 2        # copy that drained banks m%8, m%8+1
            nc.tensor.wait_ge(s_cpv if ENG[cprev] == 'v' else s_cpa,
                              POS[cprev])
        lb = m // (2 * MM_PER_G)
        nc.tensor.matmul(
            psum[:, (m % 8) * 512:(m % 8) * 512 + PAIR * FC],
            xt_sb[:, ts(m, PAIR * 128)].rearrange(
                "k (two q) -> k two q", two=2),
            rhs_sb[:, ts(lb, PAIR * PAIR * FC)].rearrange(
                "k (two n) -> k two n", two=2),
            start=True, stop=True, perf_mode=DR,
        ).then_inc(s_pe, 1)

    # ---- DVE / ACT: psum -> fp16 stage copies, strict alternation ----
    for eng, sem, tag in ((nc.vector, s_cpv, 'v'), (nc.scalar, s_cpa, 'a')):
        for c in range(NCOPY):
            if ENG[c] != tag:
                continue
            m0, nmm = COPIES[c]
            eng.wait_ge(s_pe, m0 + nmm)
            src = psum_v[:, m0 % 8:m0 % 8 + nmm, :PAIR * FC]
            dst = stage[:, m0 * PAIR * FC:(m0 + nmm) * PAIR * FC].rearrange(
                "p (f w) -> p f w", f=nmm)
            if tag == 'v':
                eng.tensor_copy(out=dst, in_=src).then_inc(sem, 1)
            else:
                eng.copy(out=dst, in_=src).then_inc(sem, 1)

    # ---- SP: 16 half-group output DMAs ----
    for h in range(16):
        # wait for every copy overlapping matmuls 4h..4h+3 (one wait per
        # engine, using the latest relevant copy's position)
        lo, hi = 4 * h, 4 * h + 4
        needed = [c for c in range(NCOPY)
                  if COPIES[c][0] < hi and COPIES[c][0] + COPIES[c][1] > lo]
        for tag, sem in (('v', s_cpv), ('a', s_cpa)):
            pos = max((POS[c] for c in needed if ENG[c] == tag), default=0)
            if pos:
                nc.sync.wait_ge(sem, pos)
        nc.sync.dma_start(
            out=out4[h // 2, h % 2],
            in_=stage[:, ts(h, 8 * FC)],
        ).then_inc(s_out, 16)

    ctx.close()
    nc.finalize()
    return nc


_NC_CACHE = None
_LAST_RESULTS = None  # BassKernelResults of the most recent run (for profiling)


def kernel(z, mask, initial_grid, W_pe, b_pe, W_clip, b_clip, emb_table,
           W_final, b_final):
    global _NC_CACHE, _LAST_RESULTS
    import ml_dtypes
    from concourse import bass_utils

    f8 = ml_dtypes.float8_e4m3
    Q_all, r_all = _precompute(z, W_pe, b_pe, W_clip, b_clip, emb_table,
                               W_final, b_final)
    qch = _chunk3(Q_all)                       # stored chunk values [3, 180]
    rmax = max(np.abs(r_all).max(), 1e-30)
    rho0 = int(np.floor(np.log2(128.0 / rmax)))
    rho = [rho0, rho0 + 4, rho0 + 8]
    r_eff = np.zeros_like(r_all)
    rch = []                                   # stored r chunks [32, 180]
    for j in range(3):
        rj = _rnd8((r_all - r_eff) * 2.0 ** rho[j])
        rch.append(rj)
        r_eff = r_eff + rj * 2.0 ** -rho[j]

    X = np.asarray(initial_grid, np.float64)

    in_maps = []
    for c in range(NCORES):
        Xc = X[B_PER_CORE * c:B_PER_CORE * (c + 1)].reshape(PTS, NFEATS)
        # point p = g*2048 + j*16 + w lives at tile (g, w), psum partition j
        X4 = Xc.reshape(GROUPS, 128, TPG, NFEATS).transpose(3, 0, 2, 1)
        xch = _chunk3(X4)                      # stored chunks [3, 8, 16, 128]
        A = np.empty((KR, GROUPS, TPG, 128), np.float64)
        for k in range(NFEATS):
            for mi in range(6):
                a, b = XCH[mi], QCH[mi]
                s = 4.0 ** -(a + b)
                # chunk c_a is stored at scale 16^a; effective value c_a/16^a;
                # row product must be (c_a/16^a)*(c_b/16^b): split 4^-(a+b)
                # onto each stored side (re-rounded; exact except subnormals)
                A[6 * k + mi] = _rnd8(xch[a][k] * s)
        for j in range(3):
            A[18 + j] = 2.0 ** -rho[j]         # bias rows: exact fp8 pow2
        # matmul m covers tiles (2*(m%8), 2*(m%8)+1) of group m//8 as the
        # two DoubleRow planes
        xt_host = (A.reshape(KR, GROUPS, MM_PER_G, PAIR, 128)
                   .transpose(0, 1, 2, 3, 4)
                   .reshape(KR, MM * PAIR * 128)).astype(f8)

        rhs_host = np.zeros((KR, B_PER_CORE, PAIR, PAIR * FC), np.float64)
        for lb in range(B_PER_CORE):
            C = np.empty((KR, FC), np.float64)
            for k in range(NFEATS):
                for mi in range(6):
                    a, b = XCH[mi], QCH[mi]
                    C[6 * k + mi] = _rnd8(qch[b][k] * 4.0 ** -(a + b))
            for j in range(3):
                C[18 + j] = rch[j][B_PER_CORE * c + lb]
            for pl in range(PAIR):             # block-diagonal planes
                rhs_host[:, lb, pl, FC * pl:FC * (pl + 1)] = C
        in_maps.append({
            "xt": np.ascontiguousarray(xt_host),
            "rhs": np.ascontiguousarray(
                rhs_host.reshape(KR, B_PER_CORE * PAIR * PAIR * FC).astype(f8)),
        })

    if _NC_CACHE is None:
        _NC_CACHE = _build_bass()
    res = bass_utils.run_bass_kernel_spmd(
        _NC_CACHE, in_maps, core_ids=list(range(NCORES))
    )
    _LAST_RESULTS = res

    out = np.empty((BS, NJOINTS, NFEATS, NFRAMES), np.float32)
    for c in range(NCORES):
        out[B_PER_CORE * c:B_PER_CORE * (c + 1)] = (
            res.results[c]["out"].astype(np.float32)
            .reshape(B_PER_CORE, NJOINTS, NFEATS, NFRAMES)
        )
    return out


# revision 26
# speedup vs baseline: 1.0453x; 1.0453x over previous
"""Trainium2 Bass kernel for nn_Decoder_TRANSFORMER_14791867367496.

The reference decoder is affine in the positions: each frame step is
    pos_{t+1} = pos_t @ M + (d_t[b] + g[b,j]),   M = I + W_pe @ W3  (3x3)
(with W_final = [W1; W2; W3] split along its 768 input rows), so the whole
60-step scan has a closed form

    out[b, j, :, t] = X[b, j, :] @ Q_t + r_t[b, :]

where X = initial_grid,
    Q_t = M^t + (W_pe @ W2) @ S_t,          S_t = sum_{k<t} M^k
    r_t[b] = h @ S_t + D_t[b],              D_t = sum_{s=1..t} d_s M^{t-s}
    d_t[b] = (emb_table[t] + z @ W_clip + b_clip) @ W1
    h      = b_pe @ (W2 + W3) + b_final

All of Q/r are tiny (3x3 / per-batch 3-vectors) and are computed on the host
in float64.  The device kernel is then a single affine map per point
([3 feats -> 180 outputs] + per-batch bias) and is bandwidth/stream bound.

Device arithmetic: fp8(e4m3) DoubleRow matmuls.  Each operand is split into
three e4m3 chunks (x = x0 + x1/16 + x2/256, ~4 mantissa bits each); all
cross terms with a+b <= 2 are kept (error ~2^-12) plus 3 e4m3 bias-chunk
rows, giving a 21-row K-stack per point-tile.  Per-row power-of-two scales
keep both factors inside e4m3 range; fp8 products are exact in the f32 PSUM
accumulation (PE upcasts to e6m3/e10m10).  DoubleRow packs the two tiles of
a pair as the two K-planes, so the PE streams 2 psum columns/cycle — half
the streaming time of the bf16 formulation.  Measured end-to-end error vs
the f32 reference is ~2.1e-4 (dominated by the fp16 output rounding).

Output is written as fp16 (norm rel err 2.1e-4 vs 2e-2 budget), halving the
HBM write traffic; the host widens back to f32 while unsharding.

Schedule (per core, 4 batches x 4096 points = 128 point-tiles = 64 pair
matmuls into the 8 psum banks): PE streams matmuls; DVE+ACT drain psum in
4-bank copies (f32 -> fp16, 7/9 split by engine speed) into a flat 46KB
stage; SP issues 16 half-group output DMAs as the copies land.  Inputs are
chunked so the first matmul's operands arrive with minimum latency (2-mm
first chunk on SP ring, rhs on ACT ring).  The fixed ~6.5us NEFF epilogue
(per-engine semaphore-range walk) overlaps the output-DMA drain: the end of
the kernel is max(last DMA + epsilon, last copy + epilogue).

Sharding: data-parallel over batch — each of the 8 cores handles 4 batches.
"""

import numpy as np

BS, NFRAMES, NJOINTS, NFEATS, LATENT, CLIP = 32, 60, 4096, 3, 256, 512
NCORES = 8
B_PER_CORE = BS // NCORES                  # 4
PTS = B_PER_CORE * NJOINTS                 # 16384 points per core
NTILES = PTS // 128                        # 128 point-tiles per core
GROUPS = 8                                 # output groups (1 psum cycle each)
TPG = NTILES // GROUPS                     # 16 tiles per group
FC = NFEATS * NFRAMES                      # 180 output columns per point
KR = 21                                    # K rows: 18 products + 3 bias
PAIR = 2                                   # tiles fused per matmul (planes)
MM = NTILES // PAIR                        # 64 matmuls
MM_PER_G = TPG // PAIR                     # 8 matmuls per group
XCH = [0, 0, 1, 0, 1, 2]                   # x-chunk index per product row
QCH = [0, 1, 0, 2, 1, 0]                   # q-chunk index per product row
# psum->stage copies as (first matmul covered, n matmuls): two-bank copies
# except the last four, which are single-bank so the final copy lands right
# after the final matmul (shortens the pre-epilogue tail).  Copies strictly
# alternate DVE/ACT so the engines always overlap and the 8-bank psum gives
# 4 copies of lookahead — PE never stalls on drains.
COPIES = [(2 * _c, 2) for _c in range(30)] + [(60, 1), (61, 1), (62, 1), (63, 1)]
NCOPY = len(COPIES)
ENG = ['v' if _c % 2 == 0 else 'a' for _c in range(NCOPY)]
POS = [_c // 2 + 1 for _c in range(NCOPY)]  # 1-based position within engine


def _precompute(z, W_pe, b_pe, W_clip, b_clip, emb_table, W_final, b_final):
    """Host-side f64 computation of the closed-form coefficients.

    Returns Q_all [3, 180] and r_all [32, 180], column layout c = f*60 + t
    (matching the [.., 3, 60] innermost layout of the output)."""
    f64 = np.float64
    W_pe64 = np.asarray(W_pe, f64)
    W_fin = np.asarray(W_final, f64)
    W1, W2, W3 = W_fin[:LATENT], W_fin[LATENT:2 * LATENT], W_fin[2 * LATENT:]
    M = np.eye(3) + W_pe64 @ W3
    Gm = W_pe64 @ W2
    b_pe64 = np.asarray(b_pe, f64)
    h = b_pe64 @ W2 + b_pe64 @ W3 + np.asarray(b_final, f64)
    z_proj = np.asarray(z, f64) @ np.asarray(W_clip, f64) + np.asarray(b_clip, f64)
    d = (np.asarray(emb_table, f64)[None, :, :] + z_proj[:, None, :]) @ W1  # [32,60,3]

    Q = np.zeros((NFRAMES, 3, 3))
    R = np.zeros((NFRAMES, BS, 3))
    Q[0] = np.eye(3)
    Mt = np.eye(3)
    S = np.zeros((3, 3))
    D = np.zeros((BS, 3))
    for t in range(1, NFRAMES):
        S = S + Mt
        Mt = Mt @ M
        D = D @ M + d[:, t, :]
        Q[t] = Mt + Gm @ S
        R[t] = h @ S + D
    Q_all = Q.transpose(1, 2, 0).reshape(3, FC)     # [k, f*60+t]
    r_all = R.transpose(1, 2, 0).reshape(BS, FC)    # [b, f*60+t]
    return Q_all, r_all


def _rnd8(a):
    """Round f64 to the TRN e4m3 grid (max normal 240), back as f64."""
    import ml_dtypes
    return np.clip(a, -240.0, 240.0).astype(ml_dtypes.float8_e4m3).astype(np.float64)


def _chunk3(a, scales=(1.0, 16.0, 256.0)):
    """Three e4m3 chunks with power-of-two residual scales: a ~ sum cj/sj."""
    c0 = _rnd8(a * scales[0]) / scales[0]
    c1 = _rnd8((a - c0) * scales[1]) / scales[1]
    c2 = _rnd8((a - c0 - c1) * scales[2]) / scales[2]
    return [c0 * scales[0], c1 * scales[1], c2 * scales[2]]  # stored values


def _build_bass():
    import concourse.mybir as mybir
    from concourse import bacc
    from concourse.bass import ts

    f32 = mybir.dt.float32
    f16 = mybir.dt.float16
    f8 = mybir.dt.float8e4
    DR = mybir.MatmulPerfMode.DoubleRow
    nc = bacc.Bacc(None, target_bir_lowering=False)
    xt = nc.dram_tensor("xt", [KR, MM * PAIR * 128], f8, kind="ExternalInput")
    rhs = nc.dram_tensor("rhs", [KR, B_PER_CORE * PAIR * PAIR * FC], f8,
                         kind="ExternalInput")
    out = nc.dram_tensor("out", [PTS, FC], f16, kind="ExternalOutput")
    # point p = g*2048 + j*16 + (h*8 + w): DMA h of group g writes 8 tiles
    out4 = out[:].rearrange("(g j h w) c -> g h j (w c)", g=GROUPS, j=128,
                            h=2, w=TPG // 2)

    from contextlib import ExitStack
    ctx = ExitStack()
    xt_sb = ctx.enter_context(
        nc.sbuf_tensor("xt_sb", [KR, MM * PAIR * 128], f8))
    rhs_sb = ctx.enter_context(
        nc.sbuf_tensor("rhs_sb", [KR, B_PER_CORE * PAIR * PAIR * FC], f8))
    stage = ctx.enter_context(
        nc.sbuf_tensor("stage", [128, NTILES * FC], f16))
    psum = ctx.enter_context(nc.psum_tensor("ps", [128, 8 * 512], f32))
    psum_v = psum[:].rearrange("p (bk w) -> p bk w", bk=8)

    s_c0a = ctx.enter_context(nc.semaphore("s_c0a"))
    s_c0b = ctx.enter_context(nc.semaphore("s_c0b"))
    s_rhs0 = ctx.enter_context(nc.semaphore("s_rhs0"))
    s_rhs123 = ctx.enter_context(nc.semaphore("s_rhs123"))
    s_ch1 = ctx.enter_context(nc.semaphore("s_ch1"))
    s_ch23 = ctx.enter_context(nc.semaphore("s_ch23"))
    s_ch47 = ctx.enter_context(nc.semaphore("s_ch47"))
    s_pe = ctx.enter_context(nc.semaphore("s_pe"))
    s_cpv = ctx.enter_context(nc.semaphore("s_cpv"))
    s_cpa = ctx.enter_context(nc.semaphore("s_cpa"))
    s_out = ctx.enter_context(nc.semaphore("s_out"))

    # ---- input DMAs ----
    # Matmul 0's operands are issued concurrently and first on the SP ring
    # (first-batch rhs — SP's sole early job) and the ACT ring (first 4
    # matmuls' xt).  Every bulk chunk is ordered or gated behind those so
    # its descriptors queue AFTER the critical ones in the shared 16
    # hardware DMA queues (per-queue FIFO, no priority) — ungated, bulk
    # descriptors delay matmul 0 by ~2us.
    RW = PAIR * PAIR * FC                  # 720 rhs columns per local batch
    nc.sync.dma_start(out=rhs_sb[:, :RW], in_=rhs[:, :RW]).then_inc(s_rhs0, 16)
    nc.scalar.dma_start(out=xt_sb[:, :1024], in_=xt[:, :1024]).then_inc(s_c0a, 16)
    nc.gpsimd.dma_start(out=xt_sb[:, 1024:2048],
                        in_=xt[:, 1024:2048]).then_inc(s_c0b, 16)
    nc.scalar.dma_start(out=rhs_sb[:, RW:], in_=rhs[:, RW:]).then_inc(s_rhs123, 16)
    nc.scalar.dma_start(out=xt_sb[:, 2048:4096],
                        in_=xt[:, 2048:4096]).then_inc(s_ch1, 16)
    nc.sync.wait_ge(s_c0a, 16)
    nc.sync.dma_start(out=xt_sb[:, 4096:8192],
                      in_=xt[:, 4096:8192]).then_inc(s_ch23, 16)
    nc.sync.dma_start(out=xt_sb[:, 8192:],
                      in_=xt[:, 8192:]).then_inc(s_ch47, 16)

    # ---- PE: 64 DoubleRow pair-matmuls ----
    for m in range(MM):
        if m == 0:
            nc.tensor.wait_ge(s_c0a, 16)
            nc.tensor.wait_ge(s_rhs0, 16)
        elif m == 4:
            nc.tensor.wait_ge(s_c0b, 16)
        elif m == 8:
            nc.tensor.wait_ge(s_ch1, 16)
        elif m == 16:
            nc.tensor.wait_ge(s_ch23, 16)
            nc.tensor.wait_ge(s_rhs123, 16)
        elif m == 32:
            nc.tensor.wait_ge(s_ch47, 16)
        if m >= 8 and m % 2 == 0:
            cprev = (m - 8) // 2        # copy that drained banks m%8, m%8+1
            nc.tensor.wait_ge(s_cpv if ENG[cprev] == 'v' else s_cpa,
                              POS[cprev])
        lb = m // (2 * MM_PER_G)
        nc.tensor.matmul(
            psum[:, (m % 8) * 512:(m % 8) * 512 + PAIR * FC],
            xt_sb[:, ts(m, PAIR * 128)].rearrange(
                "k (two q) -> k two q", two=2),
            rhs_sb[:, ts(lb, PAIR * PAIR * FC)].rearrange(
                "k (two n) -> k two n", two=2),
            start=True, stop=True, perf_mode=DR,
        ).then_inc(s_pe, 1)

    # ---- DVE / ACT: psum -> fp16 stage copies, strict alternation ----
    for eng, sem, tag in ((nc.vector, s_cpv, 'v'), (nc.scalar, s_cpa, 'a')):
        for c in range(NCOPY):
            if ENG[c] != tag:
                continue
            m0, nmm = COPIES[c]
            eng.wait_ge(s_pe, m0 + nmm)
            src = psum_v[:, m0 % 8:m0 % 8 + nmm, :PAIR * FC]
            dst = stage[:, m0 * PAIR * FC:(m0 + nmm) * PAIR * FC].rearrange(
                "p (f w) -> p f w", f=nmm)
            if tag == 'v':
                eng.tensor_copy(out=dst, in_=src).then_inc(sem, 1)
            else:
                eng.copy(out=dst, in_=src).then_inc(sem, 1)

    # ---- SP: 16 half-group output DMAs ----
    for h in range(16):
        # wait for every copy overlapping matmuls 4h..4h+3 (one wait per
        # engine, using the latest relevant copy's position)
        lo, hi = 4 * h, 4 * h + 4
        needed = [c for c in range(NCOPY)
                  if COPIES[c][0] < hi and COPIES[c][0] + COPIES[c][1] > lo]
        for tag, sem in (('v', s_cpv), ('a', s_cpa)):
            pos = max((POS[c] for c in needed if ENG[c] == tag), default=0)
            if pos:
                nc.sync.wait_ge(sem, pos)
        nc.sync.dma_start(
            out=out4[h // 2, h % 2],
            in_=stage[:, ts(h, 8 * FC)],
        ).then_inc(s_out, 16)

    ctx.close()
    nc.finalize()
    return nc


_NC_CACHE = None
_LAST_RESULTS = None  # BassKernelResults of the most recent run (for profiling)


def kernel(z, mask, initial_grid, W_pe, b_pe, W_clip, b_clip, emb_table,
           W_final, b_final):
    global _NC_CACHE, _LAST_RESULTS
    import ml_dtypes
    from concourse import bass_utils

    f8 = ml_dtypes.float8_e4m3
    Q_all, r_all = _precompute(z, W_pe, b_pe, W_clip, b_clip, emb_table,
                               W_final, b_final)
    qch = _chunk3(Q_all)                       # stored chunk values [3, 180]
    rmax = max(np.abs(r_all).max(), 1e-30)
    rho0 = int(np.floor(np.log2(128.0 / rmax)))
    rho = [rho0, rho0 + 4, rho0 + 8]
    r_eff = np.zeros_like(r_all)
    rch = []                                   # stored r chunks [32, 180]
    for j in range(3):
        rj = _rnd8((r_all - r_eff) * 2.0 ** rho[j])
        rch.append(rj)
        r_eff = r_eff + rj * 2.0 ** -rho[j]

    X = np.asarray(initial_grid, np.float64)

    in_maps = []
    for c in range(NCORES):
        Xc = X[B_PER_CORE * c:B_PER_CORE * (c + 1)].reshape(PTS, NFEATS)
        # point p = g*2048 + j*16 + w lives at tile (g, w), psum partition j
        X4 = Xc.reshape(GROUPS, 128, TPG, NFEATS).transpose(3, 0, 2, 1)
        xch = _chunk3(X4)                      # stored chunks [3, 8, 16, 128]
        A = np.empty((KR, GROUPS, TPG, 128), np.float64)
        for k in range(NFEATS):
            for mi in range(6):
                a, b = XCH[mi], QCH[mi]
                s = 4.0 ** -(a + b)
                # chunk c_a is stored at scale 16^a; effective value c_a/16^a;
                # row product must be (c_a/16^a)*(c_b/16^b): split 4^-(a+b)
                # onto each stored side (re-rounded; exact except subnormals)
                A[6 * k + mi] = _rnd8(xch[a][k] * s)
        for j in range(3):
            A[18 + j] = 2.0 ** -rho[j]         # bias rows: exact fp8 pow2
        # matmul m covers tiles (2*(m%8), 2*(m%8)+1) of group m//8 as the
        # two DoubleRow planes
        xt_host = (A.reshape(KR, GROUPS, MM_PER_G, PAIR, 128)
                   .transpose(0, 1, 2, 3, 4)
                   .reshape(KR, MM * PAIR * 128)).astype(f8)

        rhs_host = np.zeros((KR, B_PER_CORE, PAIR, PAIR * FC), np.float64)
        for lb in range(B_PER_CORE):
            C = np.empty((KR, FC), np.float64)
            for k in range(NFEATS):
                for mi in range(6):
                    a, b = XCH[mi], QCH[mi]
                    C[6 * k + mi] = _rnd8(qch[b][k] * 4.0 ** -(a + b))
            for j in range(3):
                C[18 + j] = rch[j][B_PER_CORE * c + lb]
            for pl in range(PAIR):             # block-diagonal planes
                rhs_host[:, lb, pl, FC * pl:FC * (pl + 1)] = C
        in_maps.append({
            "xt": np.ascontiguousarray(xt_host),
            "rhs": np.ascontiguousarray(
                rhs_host.reshape(KR, B_PER_CORE * PAIR * PAIR * FC).astype(f8)),
        })

    if _NC_CACHE is None:
        _NC_CACHE = _build_bass()
    res = bass_utils.run_bass_kernel_spmd(
        _NC_CACHE, in_maps, core_ids=list(range(NCORES))
    )
    _LAST_RESULTS = res

    out = np.empty((BS, NJOINTS, NFEATS, NFRAMES), np.float32)
    for c in range(NCORES):
        out[B_PER_CORE * c:B_PER_CORE * (c + 1)] = (
            res.results[c]["out"].astype(np.float32)
            .reshape(B_PER_CORE, NJOINTS, NFEATS, NFRAMES)
        )
    return out
